# revision 65
# baseline (speedup 1.0000x reference)
"""GNN max-pool message passing kernel for 8 Trainium2 NeuronCores.

Problem: out[n] = max_k s_feats[neighbor_indices[n, k]]  (N=50000, K=32, D=128)

Strategy: data-parallel over destination nodes per the sharding hint;
s_feats is replicated into every core's HBM (bf16; tolerance is 2e-2 and
bf16 rounding is ~4e-3) and each core handles 6250 destination nodes.

The gather runs on InstDMAGatherAnt (SWDGE). Measured laws on real HW:
  - The Q7 cluster's descriptor-emission loop costs ~2.1 ns per index
    POSITION aggregate (positions = ceil(num_idxs/128)*128 per call),
    independent of elem_size (up to 16 KB/descriptor), queue count, or
    single_packet. Kernel time ~= head + positions*2.1ns + tail.
  - Mixing calls of different elem_size across the four SWDGE queues
    degrades the rate to ~2.4-3.3 ns/pos; uniform-size phases restore it.

So the optimization is INDEX-COUNT COMPRESSION ("gpair" variant): one
512 B descriptor can fetch TWO neighbor rows if they are adjacent under a
host-chosen table permutation. The host runs R=3 rounds of a greedy
max-weight path-forest over neighbor co-occurrence pairs (round r+1 on
the rows left uncovered by round r), giving permutations pi_0..pi_2 and
per-node pair lists. Pair probes read row j of a sliding-window pair
table ptable_r[j] = [s[pi_r[j]], s[pi_r[j+1]]] (elem 256); leftover rows
are single probes into the main table s[pi_0] (elem 128). This removes
~34% of index positions (~200k -> ~132k per core).

Scheduling: the gather grid needs a uniform per-chunk block count, so
nodes are re-bucketed into chunks by their per-round pair counts
(lexicographic sort) and chunk c uses P_r[c] = min over chunk nodes and
cores; dropped pairs fall back to singles. Calls are merged ACROSS
chunks (segments of a call may span chunks) into uniform sizes (8 blocks
for pairs, 16 for singles) and issued in uniform phases: pairs round 0,
1, 2, then singles. Per-chunk partial maxes are combined as streams
complete; trailing-negative trim is defused by reordering each chunk's
slot-127 node lists so every call's last index is non-negative.

The K-reduction is a tensor_tensor(max) binary tree over contiguous bf16
slices (TensorReduce has NO DVE perf mode; tensor_max on packed 2-byte
data runs in 2x_1p mode at 0.5 cyc/elem). Output stays bf16 on HW
(exact) and is converted to f32 on the host, which also un-permutes the
node order.

History (8 cores, HW exec): f32 one-row-per-desc 489 us -> bf16 480 ->
pairs v1 443 -> phase-separated 418 -> uniform stage tiles 376 ->
3-round pairs 335 -> 10-round compact tables 297 -> grouped tables +
13 rounds 288 -> ucode preload + tail reorder 278 -> 16-block pair
calls 276 us (bit-exact vs the bf16-rounded reference, rel err 3.0e-3
against f32).
"""

import numpy as np

N_NODES = 50000
K = 32
D = 128
N_CORES = 8
P = 128
NODES_PER_CORE = N_NODES // N_CORES  # 6250
SLOTS = (NODES_PER_CORE + P - 1) // P  # 49
PADDED = P * SLOTS  # 6272
CHUNKS = PADDED // P  # 49 chunks of 128 nodes

VARIANT = "gpair"

_nc_cache = {}

# Pairing rounds (one permutation + pair table each). Per-round per-node
# pair-count caps level the counts so the per-chunk min-capping keeps
# ~90% of the pairs (uncapped greedy loses ~25% to chunk minima).
GPR_CAPS = (3, 3, 2, 2, 2, 2, 2, 2, 2, 2, 2, 2, 2)
GPR_ROUNDS = len(GPR_CAPS)
GPR_STORE_GROUP = 2
GPR_CALL_BLOCKS = 16  # gather blocks per merged single call
GPR_PAIR_CALL_BLOCKS = 16  # pair calls: 8 KB stage tiles, fewer call overheads
# Pairs are compact-indexed per GROUP of chunks (all rounds together) so a
# chunk's pairs form ONE contiguous gather segment -> one big DVE tree per
# chunk instead of one per round (per-op overhead dominates the DVE).
GPR_GROUPS = 4
GPR_PAIR_CAP = 32768  # compact per-group pair-table capacity (rows)
GPR_SING_CAP = 24576  # compact single-table capacity (rows)


# ----------------------------------------------------------- host: pairs ---
def _gpair_path_forest(cand_sets, seed):
    """Greedy max-weight path forest over co-occurrence pairs of the given
    per-node row lists (list of int arrays). Returns pi (permutation of all
    N_NODES rows) maximizing per-set adjacent pairs."""
    rng = np.random.default_rng(seed)
    pairs = []
    for r in cand_sets:
        n = len(r)
        if n < 2:
            continue
        i, j = np.triu_indices(n, 1)
        pairs.append(np.stack([r[i], r[j]], axis=1))
    if not pairs:
        return np.arange(N_NODES, dtype=np.int32)
    pairs = np.concatenate(pairs, axis=0)
    pairs = np.sort(pairs, axis=1)
    pairs = pairs[pairs[:, 0] != pairs[:, 1]]
    pu, counts = np.unique(
        pairs[:, 0].astype(np.int64) * N_NODES + pairs[:, 1], return_counts=True
    )
    u = (pu // N_NODES).astype(np.int32)
    v = (pu % N_NODES).astype(np.int32)
    order = np.lexsort((rng.random(len(u)), -counts))
    u, v = u[order], v[order]
    deg = np.zeros(N_NODES, np.int8)
    parent = np.arange(N_NODES, dtype=np.int32)

    def find(x):
        while parent[x] != x:
            parent[x] = parent[parent[x]]
            x = parent[x]
        return x

    adj = [[] for _ in range(N_NODES)]
    for uu, vv in zip(u.tolist(), v.tolist()):
        if deg[uu] >= 2 or deg[vv] >= 2:
            continue
        ru, rv = find(uu), find(vv)
        if ru == rv:
            continue
        parent[ru] = rv
        deg[uu] += 1
        deg[vv] += 1
        adj[uu].append(vv)
        adj[vv].append(uu)
    visited = np.zeros(N_NODES, bool)
    pi = []
    for s in range(N_NODES):
        if visited[s] or len(adj[s]) == 2:
            continue
        cur, prev = s, -1
        while True:
            pi.append(cur)
            visited[cur] = True
            nxt = [x for x in adj[cur] if x != prev and not visited[x]]
            if not nxt:
                break
            prev, cur = cur, nxt[0]
    for s in range(N_NODES):
        if not visited[s]:
            pi.append(s)
    pi = np.asarray(pi, np.int32)
    assert len(pi) == N_NODES
    return pi


def _gpair_phase1(sets):
    """Per-core multi-round pairing.

    Returns dict with:
      pis[r]: permutation per round
      pos0: row -> position in pi_0
      pair_pos[r]: per node, array of pi_r start positions of its pairs
      pair_rows[r]: per node, [p, 2] rows of those pairs
      rows_left: per node, rows not covered by any round
      pn: [M, R] per-node pair counts
    """
    m = len(sets)
    rows_left = [sets[i].astype(np.int32) for i in range(m)]
    pis, pair_pos, pair_rows = [], [], []
    pn = np.zeros((m, GPR_ROUNDS), np.int32)
    for rnd in range(GPR_ROUNDS):
        pi = _gpair_path_forest(rows_left, seed=rnd)
        pos = np.empty(N_NODES, np.int64)
        pos[pi] = np.arange(N_NODES)
        pp_r, prow_r = [], []
        new_left = []
        for i in range(m):
            r = rows_left[i]
            if len(r) < 2:
                pp_r.append(np.empty(0, np.int32))
                prow_r.append(np.empty((0, 2), np.int32))
                new_left.append(r)
                continue
            pr = np.sort(pos[r]).astype(np.int64)
            starts = []
            j = 0
            taken = np.zeros(len(r), bool)
            while j < len(r) - 1 and len(starts) < GPR_CAPS[rnd]:
                if pr[j + 1] == pr[j] + 1:
                    starts.append(pr[j])
                    taken[j] = taken[j + 1] = True
                    j += 2
                else:
                    j += 1
            starts = np.asarray(starts, np.int64)
            pp_r.append(starts.astype(np.int32))
            prow_r.append(
                np.stack([pi[starts], pi[starts + 1]], axis=1).astype(np.int32)
                if len(starts)
                else np.empty((0, 2), np.int32)
            )
            pn[i, rnd] = len(starts)
            new_left.append(pi[pr[~taken]].astype(np.int32))
        rows_left = new_left
        pis.append(pi)
        pair_pos.append(pp_r)
        pair_rows.append(prow_r)
    pos0 = np.empty(N_NODES, np.int64)
    pos0[pis[0]] = np.arange(N_NODES)
    return {
        "pis": pis,
        "pos0": pos0,
        "pair_pos": pair_pos,
        "pair_rows": pair_rows,
        "rows_left": rows_left,
        "pn": pn,
    }


# ------------------------------------------------------------- call plan ---
def _gpair_groups(P_scheds):
    """Per-chunk group id, balancing total pair instances per group (each
    group's distinct pairs must fit the 32768-row compact table)."""
    per_chunk = [
        P * sum(P_scheds[r][c] for r in range(GPR_ROUNDS))
        for c in range(CHUNKS)
    ]
    total = sum(per_chunk)
    groups = []
    acc = 0
    for c in range(CHUNKS):
        g = min(int(acc * GPR_GROUPS / max(total, 1)), GPR_GROUPS - 1)
        groups.append(g)
        acc += per_chunk[c]
    return groups


def _gpair_call_plan(P_scheds):
    """Merged cross-chunk call plan, a pure function of the schedule.

    P_scheds: tuple of GPR_ROUNDS tuples of per-chunk pair counts.
    Streams: one pair stream per chunk GROUP (a chunk's pairs from all
    rounds are contiguous in its group's compact table; uniform 8-block
    calls, elem 256), then singles (uniform 16-block calls, elem 128).
    Uniform phases keep the Q7 emission at ~2.1 ns/position.

    Returns list of calls with keys stream (group index, or -1 for
    singles), blocks, segs=[(chunk, off_in_chunk, nblocks), ...]."""
    groups = _gpair_groups(P_scheds)
    plan = []
    for g in range(GPR_GROUPS):
        blocks = []
        for c in range(CHUNKS):
            if groups[c] != g:
                continue
            n = sum(P_scheds[r][c] for r in range(GPR_ROUNDS))
            blocks += [(c, o) for o in range(n)]
        for i in range(0, len(blocks), GPR_PAIR_CALL_BLOCKS):
            chunkb = blocks[i : i + GPR_PAIR_CALL_BLOCKS]
            segs = []
            for ch, off in chunkb:
                if segs and segs[-1][0] == ch:
                    segs[-1] = (ch, segs[-1][1], segs[-1][2] + 1)
                else:
                    segs.append((ch, off, 1))
            plan.append({"stream": g, "blocks": len(chunkb), "segs": segs})
    # the last chunk (pads + low-pair nodes) has the most single blocks:
    # emit it FIRST so the kernel tail isn't gated on its big reduce
    blocks = []
    for c in [CHUNKS - 1] + list(range(CHUNKS - 1)):
        n = K - 2 * sum(P_scheds[r][c] for r in range(GPR_ROUNDS))
        blocks += [(c, o) for o in range(n)]
    for i in range(0, len(blocks), GPR_CALL_BLOCKS):
        chunkb = blocks[i : i + GPR_CALL_BLOCKS]
        segs = []
        for ch, off in chunkb:
            if segs and segs[-1][0] == ch:
                segs[-1] = (ch, segs[-1][1], segs[-1][2] + 1)
            else:
                segs.append((ch, off, 1))
        plan.append({"stream": -1, "blocks": len(chunkb), "segs": segs})
    return plan


def _gpair_phase2(core_data, P_scheds):
    """Per-core: order nodes, build the merged-call idx array with COMPACT
    per-stream indexing: each stream's used pair-starts (or single
    positions) get ids 0..U-1 (U < 32768, so every int16 index is
    non-negative and the trailing-negative trim can never fire).

    Returns (idx array [128, total_slots] int16, node order, used):
    used[stream] = array of pi positions in id order (pair starts for
    pair streams, pi_0 positions for singles)."""
    pn = core_data["pn"]
    m = len(pn)
    order = np.lexsort(
        tuple(-pn[:, r] for r in reversed(range(GPR_ROUNDS)))
    ).astype(np.int32)
    order_pad = np.concatenate([order, np.full(PADDED - m, -1, np.int32)])
    plan = _gpair_call_plan(P_scheds)
    idmaps = {s: {} for s in list(range(GPR_GROUPS)) + [-1]}

    def to_id(stream, key):
        d = idmaps[stream]
        i = d.get(key)
        if i is None:
            i = len(d)
            d[key] = i
        return i

    groups = _gpair_groups(P_scheds)
    lists_by_chunk = []  # per chunk: {group: pair ids [P, n], -1: single ids}
    for c in range(CHUNKS):
        g = groups[c]
        caps = [P_scheds[r][c] for r in range(GPR_ROUNDS)]
        s_c = K - 2 * sum(caps)
        nodes = order_pad[c * P : (c + 1) * P]
        lists = {
            g: np.zeros((P, sum(caps)), np.int32),
            -1: np.zeros((P, s_c), np.int32),
        }
        for sl in range(P):
            n = nodes[sl]
            if n < 0:
                continue  # pads keep id 0: harmless duplicate reads
            extra_rows = []
            ids = []
            for r in range(GPR_ROUNDS):
                pp = core_data["pair_pos"][r][n]
                ids += [to_id(g, (r, int(p))) for p in pp[: caps[r]]]
                if len(pp) > caps[r]:
                    extra_rows.append(
                        core_data["pair_rows"][r][n][caps[r] :].reshape(-1)
                    )
            lists[g][sl] = ids
            sing_rows = np.concatenate(
                [core_data["rows_left"][n]] + extra_rows
            ) if extra_rows else core_data["rows_left"][n]
            assert len(sing_rows) == s_c, (c, sl, len(sing_rows), s_c)
            lists[-1][sl] = [
                to_id(-1, int(p)) for p in core_data["pos0"][sing_rows]
            ]
        lists_by_chunk.append(lists)
    used = {}
    for s, d in idmaps.items():
        cap = GPR_SING_CAP if s == -1 else GPR_PAIR_CAP
        assert len(d) <= cap, (s, len(d))
        u = [0] * max(len(d), 1)
        for key, i in d.items():
            u[i] = key
        used[s] = u
    all_vals = []
    for call in plan:
        s = call["stream"]
        for ch, off, nb in call["segs"]:
            all_vals.append(
                lists_by_chunk[ch][s][:, off : off + nb]
                .T.astype(np.int16)
                .reshape(-1)
            )
    flat = np.concatenate(all_vals)
    lanes = flat.reshape(-1, 16).T
    full = np.tile(np.ascontiguousarray(lanes), (8, 1))
    return full, order_pad, used


def _prep_gpair(s_feats, neighbor_indices):
    import ml_dtypes

    s = np.ascontiguousarray(np.asarray(s_feats), dtype=np.float32).astype(
        ml_dtypes.bfloat16
    )
    nb = np.asarray(neighbor_indices)
    cores = []
    for core in range(N_CORES):
        sets = nb[core * NODES_PER_CORE : (core + 1) * NODES_PER_CORE].astype(
            np.int32
        )
        cores.append(_gpair_phase1(sets))
    # shared schedule: per chunk, per round, min pair count across cores
    # after the lexicographic node sort; pad chunks get 0
    sorted_pn = []
    for cdat in cores:
        pn = cdat["pn"]
        o = np.lexsort(tuple(-pn[:, r] for r in reversed(range(GPR_ROUNDS))))
        sorted_pn.append(pn[o])
    P_scheds = []
    for r in range(GPR_ROUNDS):
        ps = []
        for c in range(CHUNKS):
            if (c + 1) * P > NODES_PER_CORE:
                ps.append(0)
                continue
            lo, hi = c * P, (c + 1) * P
            ps.append(min(int(sp[lo:hi, r].min()) for sp in sorted_pn))
        P_scheds.append(tuple(ps))
    P_scheds = tuple(P_scheds)
    in_maps = []
    orders = []
    for core in range(N_CORES):
        idx_full, order_pad, used = _gpair_phase2(cores[core], P_scheds)
        tabs = {"idx": idx_full}
        sing = np.asarray(used[-1], np.int64)
        t = np.zeros((GPR_SING_CAP, D), s.dtype)
        t[: len(sing)] = s[cores[core]["pis"][0][sing]]
        tabs["table"] = t
        pis = cores[core]["pis"]
        for g in range(GPR_GROUPS):
            keys = used[g]
            pt = np.zeros((GPR_PAIR_CAP, 2 * D), s.dtype)
            if keys and isinstance(keys[0], tuple):
                rr_ = np.asarray([k[0] for k in keys])
                st_ = np.asarray([k[1] for k in keys], np.int64)
                for r in range(GPR_ROUNDS):
                    sel = rr_ == r
                    if not sel.any():
                        continue
                    rows = np.nonzero(sel)[0]
                    pt[rows, :D] = s[pis[r][st_[sel]]]
                    pt[rows, D:] = s[pis[r][st_[sel] + 1]]
            tabs[f"gtable{g}"] = pt
        in_maps.append(tabs)
        orders.append(order_pad)
    return in_maps, P_scheds, orders


# ---------------------------------------------------------------- kernel ---
def _build_nc_gpair(P_scheds):
    import concourse.bacc as bacc
    import concourse.mybir as mybir
    import concourse.tile as tile
    from concourse import library_config

    nc = bacc.Bacc(
        "TRN2", target_bir_lowering=False, debug=False,
        dynamic_dma_scratch_size=49152, num_swdge_queues=4,
    )
    table = nc.dram_tensor(
        "table", [GPR_SING_CAP, D], mybir.dt.bfloat16, kind="ExternalInput"
    ).ap()
    ptables = [
        nc.dram_tensor(
            f"gtable{g}", [GPR_PAIR_CAP, 2 * D], mybir.dt.bfloat16,
            kind="ExternalInput",
        ).ap()
        for g in range(GPR_GROUPS)
    ]
    plan = _gpair_call_plan(P_scheds)
    total_slots = sum(call["blocks"] * P // 16 for call in plan)
    idx = nc.dram_tensor(
        "idx", [P, total_slots], mybir.dt.int16, kind="ExternalInput"
    ).ap()
    out = nc.dram_tensor(
        "out", [PADDED, D], mybir.dt.bfloat16, kind="ExternalOutput"
    ).ap()

    with tile.TileContext(nc) as tc:
        with (
            tc.tile_pool(name="pool", bufs=1) as pool,
            tc.tile_pool(name="stage", bufs=9) as stage_pool,
            tc.tile_pool(name="tmp", bufs=8) as tmp_pool,
            tc.tile_pool(name="parts", bufs=64) as part_pool,
        ):
            # preload the Q7 ucode library so its IRAM load overlaps the
            # idx DMA instead of delaying the first gather
            nc.gpsimd.load_library(library_config.mlp)
            idx_sb = pool.tile([P, total_slots], mybir.dt.int16, name="idx_sb")
            head_cols = min(total_slots, 256)
            nc.sync.dma_start(out=idx_sb[:, :head_cols], in_=idx[:, :head_cols])
            if head_cols < total_slots:
                nc.sync.dma_start(
                    out=idx_sb[:, head_cols:], in_=idx[:, head_cols:]
                )

            res = pool.tile([P, CHUNKS * D], mybir.dt.bfloat16, name="res")
            out_view = out.rearrange("(c p) d -> p c d", p=P)
            res_view = res[:, :].rearrange("p (c d) -> p c d", d=D)

            TMP_ELEMS = GPR_CALL_BLOCKS * D  # pair trees reach 32 width-D blocks

            def tree_reduce(st, start_elems, wblocks):
                """Max-reduce wblocks width-D blocks at st[:, start_elems:]
                to one [P, D] block. Returns (tile, offset)."""
                stragglers = []
                cur, cur_off, n = st, start_elems, wblocks
                while n > 1:
                    h = n // 2
                    if n % 2:
                        stragglers.append((cur, cur_off + (n - 1) * D))
                    if h == 1:
                        dst = part_pool.tile(
                            [P, D], mybir.dt.bfloat16, tag="pt", name="tr1"
                        )
                    else:
                        dst = tmp_pool.tile(
                            [P, TMP_ELEMS], mybir.dt.bfloat16, tag="tmp",
                            name="tr",
                        )
                    nc.vector.tensor_max(
                        out=dst[:, : h * D],
                        in0=cur[:, cur_off : cur_off + h * D],
                        in1=cur[:, cur_off + h * D : cur_off + 2 * h * D],
                    )
                    cur, cur_off, n = dst, 0, h
                for sg, off in stragglers:
                    dst = part_pool.tile(
                        [P, D], mybir.dt.bfloat16, tag="pt", name="sg"
                    )
                    nc.vector.tensor_max(
                        out=dst[:, :],
                        in0=cur[:, cur_off : cur_off + D],
                        in1=sg[:, off : off + D],
                    )
                    cur, cur_off = dst, 0
                return cur, cur_off

            # per chunk, per stream: expected segment count
            exp_s = {}
            for call in plan:
                for ch, _o, _nb in call["segs"]:
                    exp_s[(ch, call["stream"])] = (
                        exp_s.get((ch, call["stream"]), 0) + 1
                    )
            got_s = {k: 0 for k in exp_s}
            n_streams_left = [0] * CHUNKS
            for (ch, _s), _v in exp_s.items():
                n_streams_left[ch] += 1
            chunk_partials = [[] for _ in range(CHUNKS)]
            done = [False] * CHUNKS
            stored_to = 0

            def collapse(ch, sink=None):
                ps_ = chunk_partials[ch]
                if sink is None and len(ps_) <= 1:
                    return
                while len(ps_) > 2:
                    (t0, o0), (t1, o1) = ps_[0], ps_[1]
                    pt = part_pool.tile(
                        [P, D], mybir.dt.bfloat16, tag="pt", name="cl"
                    )
                    nc.vector.tensor_max(
                        out=pt[:, :],
                        in0=t0[:, o0 : o0 + D],
                        in1=t1[:, o1 : o1 + D],
                    )
                    ps_ = [(pt, 0)] + ps_[2:]
                if sink is not None:
                    if len(ps_) == 1:
                        (t0, o0) = ps_[0]
                        nc.vector.tensor_max(
                            out=sink,
                            in0=t0[:, o0 : o0 + D],
                            in1=t0[:, o0 : o0 + D],
                        )
                    else:
                        (t0, o0), (t1, o1) = ps_[0], ps_[1]
                        nc.vector.tensor_max(
                            out=sink,
                            in0=t0[:, o0 : o0 + D],
                            in1=t1[:, o1 : o1 + D],
                        )
                    chunk_partials[ch] = []
                    return
                if len(ps_) == 2:
                    (t0, o0), (t1, o1) = ps_[0], ps_[1]
                    pt = part_pool.tile(
                        [P, D], mybir.dt.bfloat16, tag="pt", name="cl2"
                    )
                    nc.vector.tensor_max(
                        out=pt[:, :],
                        in0=t0[:, o0 : o0 + D],
                        in1=t1[:, o1 : o1 + D],
                    )
                    ps_ = [(pt, 0)]
                chunk_partials[ch] = ps_

            def flush_stores():
                nonlocal stored_to
                while stored_to < CHUNKS:
                    hi = min(stored_to + GPR_STORE_GROUP, CHUNKS)
                    if not all(done[stored_to:hi]):
                        return
                    nc.sync.dma_start(
                        out=out_view[:, stored_to:hi, :],
                        in_=res_view[:, stored_to:hi, :],
                    )
                    stored_to = hi

            rr = 0
            col = 0
            for call in plan:
                stream = call["stream"]
                ispair = stream >= 0
                b = call["blocks"]
                elem = 2 * D if ispair else D
                nidx = b * P
                slots = nidx // 16
                st = stage_pool.tile(
                    [P, GPR_PAIR_CALL_BLOCKS * 2 * D], mybir.dt.bfloat16,
                    tag="sst", name="st",
                )
                nc.gpsimd.dma_gather(
                    out_ap=st[:, : b * elem].rearrange("p (b d) -> p b d", d=elem),
                    in_ap=(ptables[stream] if ispair else table)[:, :],
                    idxs_ap=idx_sb[:, col : col + slots],
                    num_idxs=nidx,
                    num_idxs_reg=nidx,
                    elem_size=elem,
                    single_packet=False,
                    queue_num=rr % 4,
                )
                rr += 1
                col += slots
                boff = 0
                for ch, _off, nb in call["segs"]:
                    w = 2 * nb if ispair else nb
                    chunk_partials[ch].append(tree_reduce(st, boff * elem, w))
                    boff += nb
                    key = (ch, stream)
                    got_s[key] += 1
                    if got_s[key] == exp_s[key]:
                        n_streams_left[ch] -= 1
                        if n_streams_left[ch] == 0:
                            collapse(ch, sink=res[:, ch * D : (ch + 1) * D])
                            done[ch] = True
                        else:
                            # stream finished with this chunk: shrink held
                            # partials to one tile
                            collapse(ch)
                flush_stores()
            flush_stores()

    nc.compile()
    return nc


# -------------------------------------------------------------------- api ---
def run_variant(np_inputs, **run_kwargs):
    """Run the kernel; returns (full f32 output, BassKernelResults)."""
    from concourse.bass_utils import run_bass_kernel_spmd

    in_maps, P_scheds, orders = _prep_gpair(**np_inputs)
    key = ("gpair", P_scheds)
    if key not in _nc_cache:
        _nc_cache[key] = _build_nc_gpair(P_scheds)
    res = run_bass_kernel_spmd(
        _nc_cache[key], in_maps, core_ids=list(range(N_CORES)), **run_kwargs
    )
    out = np.empty((N_NODES, D), np.float32)
    for core in range(N_CORES):
        r = np.asarray(res.results[core]["out"]).astype(np.float32)
        order = orders[core]
        valid = order >= 0
        out[core * NODES_PER_CORE + order[valid]] = r[valid]
    return out, res


def kernel(s_feats, neighbor_indices):
    out, _ = run_variant(
        {"s_feats": s_feats, "neighbor_indices": neighbor_indices}
    )
    return out


# revision 66
# speedup vs baseline: 1.1918x; 1.1918x over previous
"""GNN max-pool message passing kernel for 8 Trainium2 NeuronCores.

Problem: out[n] = max_k s_feats[neighbor_indices[n, k]]  (N=50000, K=32, D=128)

Strategy: data-parallel over destination nodes per the sharding hint;
s_feats is replicated into every core's HBM (bf16; tolerance is 2e-2 and
bf16 rounding is ~4e-3) and each core handles 6250 destination nodes.

The gather runs on InstDMAGatherAnt (SWDGE). Measured laws on real HW:
  - The Q7 cluster's descriptor-emission loop costs ~2.1 ns per index
    POSITION aggregate (positions = ceil(num_idxs/128)*128 per call),
    independent of elem_size (up to 16 KB/descriptor), queue count, or
    single_packet. Kernel time ~= head + positions*2.1ns + tail.
  - Mixing calls of different elem_size across the four SWDGE queues
    degrades the rate to ~2.4-3.3 ns/pos; uniform-size phases restore it.

So the optimization is INDEX-COUNT COMPRESSION ("gpair" variant): one
512 B descriptor can fetch TWO neighbor rows if they are adjacent under a
host-chosen table permutation. The host runs R=3 rounds of a greedy
max-weight path-forest over neighbor co-occurrence pairs (round r+1 on
the rows left uncovered by round r), giving permutations pi_0..pi_2 and
per-node pair lists. Pair probes read row j of a sliding-window pair
table ptable_r[j] = [s[pi_r[j]], s[pi_r[j+1]]] (elem 256); leftover rows
are single probes into the main table s[pi_0] (elem 128). This removes
~34% of index positions (~200k -> ~132k per core).

Scheduling: the gather grid needs a uniform per-chunk block count, so
nodes are re-bucketed into chunks by their per-round pair counts
(lexicographic sort) and chunk c uses P_r[c] = min over chunk nodes and
cores; dropped pairs fall back to singles. Calls are merged ACROSS
chunks (segments of a call may span chunks) into uniform sizes (8 blocks
for pairs, 16 for singles) and issued in uniform phases: pairs round 0,
1, 2, then singles. Per-chunk partial maxes are combined as streams
complete; trailing-negative trim is defused by reordering each chunk's
slot-127 node lists so every call's last index is non-negative.

The K-reduction is a tensor_tensor(max) binary tree over contiguous bf16
slices (TensorReduce has NO DVE perf mode; tensor_max on packed 2-byte
data runs in 2x_1p mode at 0.5 cyc/elem). Output stays bf16 on HW
(exact) and is converted to f32 on the host, which also un-permutes the
node order.

History (8 cores, HW exec): f32 one-row-per-desc 489 us -> bf16 480 ->
pairs v1 443 -> phase-separated 418 -> uniform stage tiles 376 ->
3-round pairs 335 -> 10-round compact tables 297 -> grouped tables +
13 rounds 288 -> ucode preload + tail reorder 278 -> 16-block pair
calls 276 us (bit-exact vs the bf16-rounded reference, rel err 3.0e-3
against f32).
"""

import numpy as np

N_NODES = 50000
K = 32
D = 128
N_CORES = 8
P = 128
NODES_PER_CORE = N_NODES // N_CORES  # 6250
SLOTS = (NODES_PER_CORE + P - 1) // P  # 49
PADDED = P * SLOTS  # 6272
CHUNKS = PADDED // P  # 49 chunks of 128 nodes

VARIANT = "gpair"

_nc_cache = {}

# Pairing rounds (one permutation + pair table each). Per-round per-node
# pair-count caps level the counts so the per-chunk min-capping keeps
# ~90% of the pairs (uncapped greedy loses ~25% to chunk minima).
GPR_CAPS = (3, 3, 2, 2, 2, 2, 2, 2, 2, 2, 2, 2, 2)
GPR_ROUNDS = len(GPR_CAPS)
GPR_STORE_GROUP = 4
GPR_CALL_BLOCKS = 16  # gather blocks per merged single call
GPR_PAIR_CALL_BLOCKS = 16  # pair calls: 8 KB stage tiles, fewer call overheads
# Pairs are compact-indexed per GROUP of chunks (all rounds together) so a
# chunk's pairs form ONE contiguous gather segment -> one big DVE tree per
# chunk instead of one per round (per-op overhead dominates the DVE).
GPR_GROUPS = 4
GPR_PAIR_CAP = 32768  # compact per-group pair-table capacity (rows)
GPR_SING_CAP = 24576  # compact single-table capacity (rows)


# ----------------------------------------------------------- host: pairs ---
def _gpair_path_forest(cand_sets, seed):
    """Greedy max-weight path forest over co-occurrence pairs of the given
    per-node row lists (list of int arrays). Returns pi (permutation of all
    N_NODES rows) maximizing per-set adjacent pairs."""
    rng = np.random.default_rng(seed)
    pairs = []
    for r in cand_sets:
        n = len(r)
        if n < 2:
            continue
        i, j = np.triu_indices(n, 1)
        pairs.append(np.stack([r[i], r[j]], axis=1))
    if not pairs:
        return np.arange(N_NODES, dtype=np.int32)
    pairs = np.concatenate(pairs, axis=0)
    pairs = np.sort(pairs, axis=1)
    pairs = pairs[pairs[:, 0] != pairs[:, 1]]
    pu, counts = np.unique(
        pairs[:, 0].astype(np.int64) * N_NODES + pairs[:, 1], return_counts=True
    )
    u = (pu // N_NODES).astype(np.int32)
    v = (pu % N_NODES).astype(np.int32)
    order = np.lexsort((rng.random(len(u)), -counts))
    u, v = u[order], v[order]
    deg = np.zeros(N_NODES, np.int8)
    parent = np.arange(N_NODES, dtype=np.int32)

    def find(x):
        while parent[x] != x:
            parent[x] = parent[parent[x]]
            x = parent[x]
        return x

    adj = [[] for _ in range(N_NODES)]
    for uu, vv in zip(u.tolist(), v.tolist()):
        if deg[uu] >= 2 or deg[vv] >= 2:
            continue
        ru, rv = find(uu), find(vv)
        if ru == rv:
            continue
        parent[ru] = rv
        deg[uu] += 1
        deg[vv] += 1
        adj[uu].append(vv)
        adj[vv].append(uu)
    visited = np.zeros(N_NODES, bool)
    pi = []
    for s in range(N_NODES):
        if visited[s] or len(adj[s]) == 2:
            continue
        cur, prev = s, -1
        while True:
            pi.append(cur)
            visited[cur] = True
            nxt = [x for x in adj[cur] if x != prev and not visited[x]]
            if not nxt:
                break
            prev, cur = cur, nxt[0]
    for s in range(N_NODES):
        if not visited[s]:
            pi.append(s)
    pi = np.asarray(pi, np.int32)
    assert len(pi) == N_NODES
    return pi


def _gpair_phase1(sets):
    """Per-core multi-round pairing.

    Returns dict with:
      pis[r]: permutation per round
      pos0: row -> position in pi_0
      pair_pos[r]: per node, array of pi_r start positions of its pairs
      pair_rows[r]: per node, [p, 2] rows of those pairs
      rows_left: per node, rows not covered by any round
      pn: [M, R] per-node pair counts
    """
    m = len(sets)
    rows_left = [sets[i].astype(np.int32) for i in range(m)]
    pis, pair_pos, pair_rows = [], [], []
    pn = np.zeros((m, GPR_ROUNDS), np.int32)
    for rnd in range(GPR_ROUNDS):
        pi = _gpair_path_forest(rows_left, seed=rnd)
        pos = np.empty(N_NODES, np.int64)
        pos[pi] = np.arange(N_NODES)
        pp_r, prow_r = [], []
        new_left = []
        for i in range(m):
            r = rows_left[i]
            if len(r) < 2:
                pp_r.append(np.empty(0, np.int32))
                prow_r.append(np.empty((0, 2), np.int32))
                new_left.append(r)
                continue
            pr = np.sort(pos[r]).astype(np.int64)
            starts = []
            j = 0
            taken = np.zeros(len(r), bool)
            while j < len(r) - 1 and len(starts) < GPR_CAPS[rnd]:
                if pr[j + 1] == pr[j] + 1:
                    starts.append(pr[j])
                    taken[j] = taken[j + 1] = True
                    j += 2
                else:
                    j += 1
            starts = np.asarray(starts, np.int64)
            pp_r.append(starts.astype(np.int32))
            prow_r.append(
                np.stack([pi[starts], pi[starts + 1]], axis=1).astype(np.int32)
                if len(starts)
                else np.empty((0, 2), np.int32)
            )
            pn[i, rnd] = len(starts)
            new_left.append(pi[pr[~taken]].astype(np.int32))
        rows_left = new_left
        pis.append(pi)
        pair_pos.append(pp_r)
        pair_rows.append(prow_r)
    pos0 = np.empty(N_NODES, np.int64)
    pos0[pis[0]] = np.arange(N_NODES)
    return {
        "pis": pis,
        "pos0": pos0,
        "pair_pos": pair_pos,
        "pair_rows": pair_rows,
        "rows_left": rows_left,
        "pn": pn,
    }


# ------------------------------------------------------------- call plan ---
def _gpair_groups(P_scheds):
    """Per-chunk group id, balancing total pair instances per group (each
    group's distinct pairs must fit the 32768-row compact table)."""
    per_chunk = [
        P * sum(P_scheds[r][c] for r in range(GPR_ROUNDS))
        for c in range(CHUNKS)
    ]
    total = sum(per_chunk)
    groups = []
    acc = 0
    for c in range(CHUNKS):
        g = min(int(acc * GPR_GROUPS / max(total, 1)), GPR_GROUPS - 1)
        groups.append(g)
        acc += per_chunk[c]
    return groups


def _gpair_call_plan(P_scheds):
    """Merged cross-chunk call plan, a pure function of the schedule.

    P_scheds: tuple of GPR_ROUNDS tuples of per-chunk pair counts.
    Streams: one pair stream per chunk GROUP (a chunk's pairs from all
    rounds are contiguous in its group's compact table; uniform 8-block
    calls, elem 256), then singles (uniform 16-block calls, elem 128).
    Uniform phases keep the Q7 emission at ~2.1 ns/position.

    Returns list of calls with keys stream (group index, or -1 for
    singles), blocks, segs=[(chunk, off_in_chunk, nblocks), ...]."""
    groups = _gpair_groups(P_scheds)
    plan = []
    for g in range(GPR_GROUPS):
        blocks = []
        for c in range(CHUNKS):
            if groups[c] != g:
                continue
            n = sum(P_scheds[r][c] for r in range(GPR_ROUNDS))
            blocks += [(c, o) for o in range(n)]
        for i in range(0, len(blocks), GPR_PAIR_CALL_BLOCKS):
            chunkb = blocks[i : i + GPR_PAIR_CALL_BLOCKS]
            segs = []
            for ch, off in chunkb:
                if segs and segs[-1][0] == ch:
                    segs[-1] = (ch, segs[-1][1], segs[-1][2] + 1)
                else:
                    segs.append((ch, off, 1))
            plan.append({"stream": g, "blocks": len(chunkb), "segs": segs})
    # the last chunk (pads + low-pair nodes) has the most single blocks:
    # emit it FIRST so the kernel tail isn't gated on its big reduce
    blocks = []
    for c in [CHUNKS - 1] + list(range(CHUNKS - 1)):
        n = K - 2 * sum(P_scheds[r][c] for r in range(GPR_ROUNDS))
        blocks += [(c, o) for o in range(n)]
    for i in range(0, len(blocks), GPR_CALL_BLOCKS):
        chunkb = blocks[i : i + GPR_CALL_BLOCKS]
        segs = []
        for ch, off in chunkb:
            if segs and segs[-1][0] == ch:
                segs[-1] = (ch, segs[-1][1], segs[-1][2] + 1)
            else:
                segs.append((ch, off, 1))
        plan.append({"stream": -1, "blocks": len(chunkb), "segs": segs})
    return plan


def _gpair_phase2(core_data, P_scheds):
    """Per-core: order nodes, build the merged-call idx array with COMPACT
    per-stream indexing: each stream's used pair-starts (or single
    positions) get ids 0..U-1 (U < 32768, so every int16 index is
    non-negative and the trailing-negative trim can never fire).

    Returns (idx array [128, total_slots] int16, node order, used):
    used[stream] = array of pi positions in id order (pair starts for
    pair streams, pi_0 positions for singles)."""
    pn = core_data["pn"]
    m = len(pn)
    order = np.lexsort(
        tuple(-pn[:, r] for r in reversed(range(GPR_ROUNDS)))
    ).astype(np.int32)
    order_pad = np.concatenate([order, np.full(PADDED - m, -1, np.int32)])
    plan = _gpair_call_plan(P_scheds)
    idmaps = {s: {} for s in list(range(GPR_GROUPS)) + [-1]}

    def to_id(stream, key):
        d = idmaps[stream]
        i = d.get(key)
        if i is None:
            i = len(d)
            d[key] = i
        return i

    groups = _gpair_groups(P_scheds)
    lists_by_chunk = []  # per chunk: {group: pair ids [P, n], -1: single ids}
    for c in range(CHUNKS):
        g = groups[c]
        caps = [P_scheds[r][c] for r in range(GPR_ROUNDS)]
        s_c = K - 2 * sum(caps)
        nodes = order_pad[c * P : (c + 1) * P]
        lists = {
            g: np.zeros((P, sum(caps)), np.int32),
            -1: np.zeros((P, s_c), np.int32),
        }
        for sl in range(P):
            n = nodes[sl]
            if n < 0:
                continue  # pads keep id 0: harmless duplicate reads
            extra_rows = []
            ids = []
            for r in range(GPR_ROUNDS):
                pp = core_data["pair_pos"][r][n]
                ids += [to_id(g, (r, int(p))) for p in pp[: caps[r]]]
                if len(pp) > caps[r]:
                    extra_rows.append(
                        core_data["pair_rows"][r][n][caps[r] :].reshape(-1)
                    )
            lists[g][sl] = ids
            sing_rows = np.concatenate(
                [core_data["rows_left"][n]] + extra_rows
            ) if extra_rows else core_data["rows_left"][n]
            assert len(sing_rows) == s_c, (c, sl, len(sing_rows), s_c)
            lists[-1][sl] = [
                to_id(-1, int(p)) for p in core_data["pos0"][sing_rows]
            ]
        lists_by_chunk.append(lists)
    used = {}
    for s, d in idmaps.items():
        cap = GPR_SING_CAP if s == -1 else GPR_PAIR_CAP
        assert len(d) <= cap, (s, len(d))
        u = [0] * max(len(d), 1)
        for key, i in d.items():
            u[i] = key
        used[s] = u
    all_vals = []
    for call in plan:
        s = call["stream"]
        for ch, off, nb in call["segs"]:
            all_vals.append(
                lists_by_chunk[ch][s][:, off : off + nb]
                .T.astype(np.int16)
                .reshape(-1)
            )
    flat = np.concatenate(all_vals)
    lanes = flat.reshape(-1, 16).T
    full = np.tile(np.ascontiguousarray(lanes), (8, 1))
    return full, order_pad, used


def _prep_gpair(s_feats, neighbor_indices):
    import ml_dtypes

    s = np.ascontiguousarray(np.asarray(s_feats), dtype=np.float32).astype(
        ml_dtypes.bfloat16
    )
    nb = np.asarray(neighbor_indices)
    cores = []
    for core in range(N_CORES):
        sets = nb[core * NODES_PER_CORE : (core + 1) * NODES_PER_CORE].astype(
            np.int32
        )
        cores.append(_gpair_phase1(sets))
    # shared schedule: per chunk, per round, min pair count across cores
    # after the lexicographic node sort; pad chunks get 0
    sorted_pn = []
    for cdat in cores:
        pn = cdat["pn"]
        o = np.lexsort(tuple(-pn[:, r] for r in reversed(range(GPR_ROUNDS))))
        sorted_pn.append(pn[o])
    P_scheds = []
    for r in range(GPR_ROUNDS):
        ps = []
        for c in range(CHUNKS):
            if (c + 1) * P > NODES_PER_CORE:
                ps.append(0)
                continue
            lo, hi = c * P, (c + 1) * P
            ps.append(min(int(sp[lo:hi, r].min()) for sp in sorted_pn))
        P_scheds.append(tuple(ps))
    P_scheds = tuple(P_scheds)
    in_maps = []
    orders = []
    for core in range(N_CORES):
        idx_full, order_pad, used = _gpair_phase2(cores[core], P_scheds)
        tabs = {"idx": idx_full}
        sing = np.asarray(used[-1], np.int64)
        t = np.zeros((GPR_SING_CAP, D), s.dtype)
        t[: len(sing)] = s[cores[core]["pis"][0][sing]]
        tabs["table"] = t
        pis = cores[core]["pis"]
        for g in range(GPR_GROUPS):
            keys = used[g]
            pt = np.zeros((GPR_PAIR_CAP, 2 * D), s.dtype)
            if keys and isinstance(keys[0], tuple):
                rr_ = np.asarray([k[0] for k in keys])
                st_ = np.asarray([k[1] for k in keys], np.int64)
                for r in range(GPR_ROUNDS):
                    sel = rr_ == r
                    if not sel.any():
                        continue
                    rows = np.nonzero(sel)[0]
                    pt[rows, :D] = s[pis[r][st_[sel]]]
                    pt[rows, D:] = s[pis[r][st_[sel] + 1]]
            tabs[f"gtable{g}"] = pt
        in_maps.append(tabs)
        orders.append(order_pad)
    return in_maps, P_scheds, orders


# ---------------------------------------------------------------- kernel ---
def _build_nc_gpair(P_scheds):
    import concourse.bacc as bacc
    import concourse.mybir as mybir
    import concourse.tile as tile
    from concourse import library_config

    nc = bacc.Bacc(
        "TRN2", target_bir_lowering=False, debug=False,
        dynamic_dma_scratch_size=49152, num_swdge_queues=4,
    )
    table = nc.dram_tensor(
        "table", [GPR_SING_CAP, D], mybir.dt.bfloat16, kind="ExternalInput"
    ).ap()
    ptables = [
        nc.dram_tensor(
            f"gtable{g}", [GPR_PAIR_CAP, 2 * D], mybir.dt.bfloat16,
            kind="ExternalInput",
        ).ap()
        for g in range(GPR_GROUPS)
    ]
    plan = _gpair_call_plan(P_scheds)
    total_slots = sum(call["blocks"] * P // 16 for call in plan)
    idx = nc.dram_tensor(
        "idx", [P, total_slots], mybir.dt.int16, kind="ExternalInput"
    ).ap()
    out = nc.dram_tensor(
        "out", [PADDED, D], mybir.dt.bfloat16, kind="ExternalOutput"
    ).ap()

    with tile.TileContext(nc) as tc:
        with (
            tc.tile_pool(name="pool", bufs=1) as pool,
            tc.tile_pool(name="stage", bufs=9) as stage_pool,
            tc.tile_pool(name="tmp", bufs=8) as tmp_pool,
            tc.tile_pool(name="parts", bufs=64) as part_pool,
        ):
            # preload the Q7 ucode library so its IRAM load overlaps the
            # idx DMA instead of delaying the first gather
            nc.gpsimd.load_library(library_config.mlp)
            idx_sb = pool.tile([P, total_slots], mybir.dt.int16, name="idx_sb")
            head_cols = min(total_slots, 256)
            nc.sync.dma_start(out=idx_sb[:, :head_cols], in_=idx[:, :head_cols])
            if head_cols < total_slots:
                nc.sync.dma_start(
                    out=idx_sb[:, head_cols:], in_=idx[:, head_cols:]
                )

            res = pool.tile([P, CHUNKS * D], mybir.dt.bfloat16, name="res")
            out_view = out.rearrange("(c p) d -> p c d", p=P)
            res_view = res[:, :].rearrange("p (c d) -> p c d", d=D)

            TMP_ELEMS = GPR_CALL_BLOCKS * D  # pair trees reach 32 width-D blocks

            def tree_reduce(st, start_elems, wblocks):
                """Max-reduce wblocks width-D blocks at st[:, start_elems:]
                to one [P, D] block. Returns (tile, offset)."""
                stragglers = []
                cur, cur_off, n = st, start_elems, wblocks
                while n > 1:
                    h = n // 2
                    if n % 2:
                        stragglers.append((cur, cur_off + (n - 1) * D))
                    if h == 1:
                        dst = part_pool.tile(
                            [P, D], mybir.dt.bfloat16, tag="pt", name="tr1"
                        )
                    else:
                        dst = tmp_pool.tile(
                            [P, TMP_ELEMS], mybir.dt.bfloat16, tag="tmp",
                            name="tr",
                        )
                    nc.vector.tensor_max(
                        out=dst[:, : h * D],
                        in0=cur[:, cur_off : cur_off + h * D],
                        in1=cur[:, cur_off + h * D : cur_off + 2 * h * D],
                    )
                    cur, cur_off, n = dst, 0, h
                for sg, off in stragglers:
                    dst = part_pool.tile(
                        [P, D], mybir.dt.bfloat16, tag="pt", name="sg"
                    )
                    nc.vector.tensor_max(
                        out=dst[:, :],
                        in0=cur[:, cur_off : cur_off + D],
                        in1=sg[:, off : off + D],
                    )
                    cur, cur_off = dst, 0
                return cur, cur_off

            # per chunk, per stream: expected segment count
            exp_s = {}
            for call in plan:
                for ch, _o, _nb in call["segs"]:
                    exp_s[(ch, call["stream"])] = (
                        exp_s.get((ch, call["stream"]), 0) + 1
                    )
            got_s = {k: 0 for k in exp_s}
            n_streams_left = [0] * CHUNKS
            for (ch, _s), _v in exp_s.items():
                n_streams_left[ch] += 1
            chunk_partials = [[] for _ in range(CHUNKS)]
            done = [False] * CHUNKS
            stored_to = 0

            def collapse(ch, sink=None):
                ps_ = chunk_partials[ch]
                if sink is None and len(ps_) <= 1:
                    return
                while len(ps_) > 2:
                    (t0, o0), (t1, o1) = ps_[0], ps_[1]
                    pt = part_pool.tile(
                        [P, D], mybir.dt.bfloat16, tag="pt", name="cl"
                    )
                    nc.vector.tensor_max(
                        out=pt[:, :],
                        in0=t0[:, o0 : o0 + D],
                        in1=t1[:, o1 : o1 + D],
                    )
                    ps_ = [(pt, 0)] + ps_[2:]
                if sink is not None:
                    if len(ps_) == 1:
                        (t0, o0) = ps_[0]
                        nc.vector.tensor_max(
                            out=sink,
                            in0=t0[:, o0 : o0 + D],
                            in1=t0[:, o0 : o0 + D],
                        )
                    else:
                        (t0, o0), (t1, o1) = ps_[0], ps_[1]
                        nc.vector.tensor_max(
                            out=sink,
                            in0=t0[:, o0 : o0 + D],
                            in1=t1[:, o1 : o1 + D],
                        )
                    chunk_partials[ch] = []
                    return
                if len(ps_) == 2:
                    (t0, o0), (t1, o1) = ps_[0], ps_[1]
                    pt = part_pool.tile(
                        [P, D], mybir.dt.bfloat16, tag="pt", name="cl2"
                    )
                    nc.vector.tensor_max(
                        out=pt[:, :],
                        in0=t0[:, o0 : o0 + D],
                        in1=t1[:, o1 : o1 + D],
                    )
                    ps_ = [(pt, 0)]
                chunk_partials[ch] = ps_

            def flush_stores():
                nonlocal stored_to
                while stored_to < CHUNKS:
                    hi = min(stored_to + GPR_STORE_GROUP, CHUNKS)
                    if not all(done[stored_to:hi]):
                        return
                    nc.sync.dma_start(
                        out=out_view[:, stored_to:hi, :],
                        in_=res_view[:, stored_to:hi, :],
                    )
                    stored_to = hi

            rr = 0
            col = 0
            for call in plan:
                stream = call["stream"]
                ispair = stream >= 0
                b = call["blocks"]
                elem = 2 * D if ispair else D
                nidx = b * P
                slots = nidx // 16
                st = stage_pool.tile(
                    [P, GPR_PAIR_CALL_BLOCKS * 2 * D], mybir.dt.bfloat16,
                    tag="sst", name="st",
                )
                nc.gpsimd.dma_gather(
                    out_ap=st[:, : b * elem].rearrange("p (b d) -> p b d", d=elem),
                    in_ap=(ptables[stream] if ispair else table)[:, :],
                    idxs_ap=idx_sb[:, col : col + slots],
                    num_idxs=nidx,
                    num_idxs_reg=nidx,
                    elem_size=elem,
                    single_packet=False,
                    queue_num=rr % 4,
                )
                rr += 1
                col += slots
                boff = 0
                for ch, _off, nb in call["segs"]:
                    w = 2 * nb if ispair else nb
                    chunk_partials[ch].append(tree_reduce(st, boff * elem, w))
                    boff += nb
                    key = (ch, stream)
                    got_s[key] += 1
                    if got_s[key] == exp_s[key]:
                        n_streams_left[ch] -= 1
                        if n_streams_left[ch] == 0:
                            collapse(ch, sink=res[:, ch * D : (ch + 1) * D])
                            done[ch] = True
                        else:
                            # stream finished with this chunk: shrink held
                            # partials to one tile
                            collapse(ch)
                flush_stores()
            flush_stores()

    nc.compile()
    return nc


# -------------------------------------------------------------------- api ---
def run_variant(np_inputs, **run_kwargs):
    """Run the kernel; returns (full f32 output, BassKernelResults)."""
    from concourse.bass_utils import run_bass_kernel_spmd

    in_maps, P_scheds, orders = _prep_gpair(**np_inputs)
    key = ("gpair", P_scheds)
    if key not in _nc_cache:
        _nc_cache[key] = _build_nc_gpair(P_scheds)
    res = run_bass_kernel_spmd(
        _nc_cache[key], in_maps, core_ids=list(range(N_CORES)), **run_kwargs
    )
    out = np.empty((N_NODES, D), np.float32)
    for core in range(N_CORES):
        r = np.asarray(res.results[core]["out"]).astype(np.float32)
        order = orders[core]
        valid = order >= 0
        out[core * NODES_PER_CORE + order[valid]] = r[valid]
    return out, res


def kernel(s_feats, neighbor_indices):
    out, _ = run_variant(
        {"s_feats": s_feats, "neighbor_indices": neighbor_indices}
    )
    return out
